# revision 2
# baseline (speedup 1.0000x reference)
"""Trainium2 Bass kernel for DeformationTrackerBiFlowModel — G=9 K-split.

Reference math (per batch element b, per step t):
    x_t   = [prev_out (2), fin_t (3)]            (5,)
    h_t   = tanh(x_t @ W_rnn + b_rnn)            (12,)   (U_rnn is inert)
    out_t = [cp0 (2), h_t (12)] @ W_out + b_out  (2,)
    prev_out_{t+1} = out_t;  prev_out_0 = cp0

Folded recurrence (h carries the state; out is a readout):
    pre_t = h_{t-1} @ Wh + fin_t @ W1f + 1*r + cp0 @ E     Wh = Wo2 @ W1p
    h_t   = tanh(pre_t)
    out_t = cvec + h_t @ Wo2                                cvec = cp0 @ Wo1 + b_out

G=9 trajectories per column need K = 12G(h) + 3G(fin) + 1 + 2G(cp0) = 154 > 128,
so the contraction is split into two accumulating matmuls per step:
    MM1: K=108  h rows          -> Wh into h-cols, Wo2 into out-cols
    MM2: K=64   fin|ones|cp0|cvec -> W1f / r / E into h-cols, I into out-cols
psum layout M=126: pre (108) | out (18).  The cvec identity rows make the out
region hold the FINAL out values, so a single ACT per chain-step tanh's the
whole [126] tile: h rows feed the next step's rhs, and the out rows are staged
as tanh(out) (undone exactly on the host with arctanh; |out| ~ 0.16 so the
roundtrip is benign).  No DVE work at all.  ACT busy/step = C*(COLS+222)/1.2.

Batch 65536 over 8 cores; per core G*C*COLS = 9*2*456 = 8208 (8192 + pad 16).
"""

import os
from contextlib import ExitStack

import numpy as np

import concourse.mybir as mybir
import concourse.tile as tile
from concourse import bacc
from concourse.bass_utils import run_bass_kernel_spmd

B, T = 65536, 100
D_CP, D_FIN, HID = 2, 3, 12
NCORES = 8
BC = B // NCORES              # 8192 per core
G = 9                         # trajectories packed per column (block-diag)
C = int(os.environ.get("DTB_C", "2"))   # independent column chains
COLS = -(-BC // (G * C))      # batch columns per chain
BP = G * C * COLS             # padded batch per core
NH = HID * G                  # 108: h rows (rhs) / pre rows (psum)
NFIN = D_FIN * G              # 27 fin rows
K2 = NFIN + 1 + 2 * D_CP * G  # 64: fin + ones + cp0 + cvec rows
MOUT = D_CP * G               # 18 out rows
MTOT = NH + MOUT              # 126

F32 = mybir.dt.float32

_MM_CHOICES = {"bf16": mybir.dt.bfloat16, "f32r": mybir.dt.float32r, "f32": F32}
MM_DTYPE = _MM_CHOICES[os.environ.get("DTB_MM", "bf16")]
MM_NP = mybir.dt.np(MM_DTYPE)

LAST_RESULTS = None  # test.py introspects profiling info from here


def build_program(t_steps=T, g=G, c=C, cols=COLS, mm_dtype=None):
    if mm_dtype is None:
        mm_dtype = MM_DTYPE
    XDT = mm_dtype
    nh, nfin = HID * g, D_FIN * g
    k2 = nfin + 1 + 2 * D_CP * g
    mout = D_CP * g
    mtot = nh + mout
    nc = bacc.Bacc(target_bir_lowering=False)

    fin = nc.dram_tensor("fin", [t_steps, c, nfin, cols], XDT, kind="ExternalInput")
    xc = nc.dram_tensor("xc", [c, k2 - nfin, 8 * cols], XDT, kind="ExternalInput")
    w1 = nc.dram_tensor("w1", [nh, mtot], XDT, kind="ExternalInput")
    w2 = nc.dram_tensor("w2", [k2, mtot], XDT, kind="ExternalInput")
    w20 = nc.dram_tensor("w20", [k2, mtot], XDT, kind="ExternalInput")
    out = nc.dram_tensor("out", [t_steps, c, mout, cols], XDT, kind="ExternalOutput")

    tanh = mybir.ActivationFunctionType.Tanh

    with tile.TileContext(nc) as tc, ExitStack() as ctx:
        const = ctx.enter_context(tc.tile_pool(name="const", bufs=1))
        xpool = ctx.enter_context(tc.tile_pool(name="xpool", bufs=1))
        psum = ctx.enter_context(tc.tile_pool(name="psum", bufs=2, space="PSUM"))

        w1s = const.tile([nh, mtot], XDT, name="w1s")
        nc.sync.dma_start(out=w1s, in_=w1[:, :])
        w2s = const.tile([k2, mtot], XDT, name="w2s")
        nc.sync.dma_start(out=w2s, in_=w2[:, :])
        w20s = const.tile([k2, mtot], XDT, name="w20s")
        nc.sync.dma_start(out=w20s, in_=w20[:, :])

        def quad_src(ap4):
            return ap4.rearrange("t r c -> r t c")

        # Persistent per-chain tiles, 8 column-blocks each (block = t % 8 for
        # fc, (t-1) % 8 for h/out staging).  DMA instruction cost is dominated
        # by fixed per-transfer overhead, so fin is shipped 4 steps per DMA
        # and out is drained 4 steps per DMA, double-buffered across the two
        # half-ranges {0..3} / {4..7}.
        htiles, fctiles = [], []
        for ch in range(c):
            ht = xpool.tile([mtot, 8 * cols], XDT, tag=f"h{ch}", name=f"h_{ch}")
            htiles.append(ht)
            fc = xpool.tile([k2, 8 * cols], XDT, tag=f"f{ch}", name=f"f_{ch}")
            nc.sync.dma_start(out=fc[nfin:, :], in_=xc[ch])
            nc.sync.dma_start(
                out=fc[0:nfin, 0 : 4 * cols].rearrange("r (t c) -> r t c", t=4),
                in_=quad_src(fin[0:4, ch]),
            )
            fctiles.append(fc)

        for t in range(t_steps + 1):
            for ch in range(c):
                ht, fc = htiles[ch], fctiles[ch]
                fblk = t % 8
                hblk = (t - 1) % 8  # ACT_t dest; MM1_t reads (t-2) % 8
                p1 = psum.tile([mtot, cols], F32, tag=f"p{ch}", name=f"p_{ch}_{t}")
                nc.tensor.matmul(
                    p1, w20s if t == 0 else w2s,
                    fc[:, fblk * cols : (fblk + 1) * cols],
                    start=True, stop=(t == 0),
                )
                if t > 0:
                    rb = (t - 2) % 8
                    nc.tensor.matmul(
                        p1, w1s, ht[0:nh, rb * cols : (rb + 1) * cols],
                        start=False, stop=True,
                    )
                # One ACT per chain-step: tanh over pre AND out rows; h lands
                # in the next step's rhs block, tanh(out) is staged for DMA.
                nc.scalar.activation(
                    ht[:, hblk * cols : (hblk + 1) * cols], p1[:, :], tanh
                )
                # Drain out_{t-4..t-1} (blocks hblk-3..hblk) every 4 steps.
                if t >= 4 and t % 4 == 0:
                    b0 = (t - 4) % 8
                    nc.gpsimd.dma_start(
                        out=quad_src(out[t - 4 : t, ch]),
                        in_=ht[nh:, b0 * cols : (b0 + 4) * cols].rearrange(
                            "r (t c) -> r t c", t=4
                        ),
                    )
                # Prefetch fin 4 steps per DMA, two steps ahead.
                s0 = t + 2
                if s0 % 4 == 0 and s0 < t_steps:
                    bs = s0 % 8
                    nc.sync.dma_start(
                        out=fc[0:nfin, bs * cols : (bs + 4) * cols].rearrange(
                            "r (t c) -> r t c", t=4
                        ),
                        in_=quad_src(fin[s0 : s0 + 4, ch]),
                    )
    nc.compile()
    return nc


def build_packed_weights(W_rnn, W_out, b_rnn, b_out, g=G):
    W_rnn = np.asarray(W_rnn, np.float32)
    W_out = np.asarray(W_out, np.float32)
    b_rnn = np.asarray(b_rnn, np.float32)
    b_out = np.asarray(b_out, np.float32)
    W1p, W1f = W_rnn[:D_CP], W_rnn[D_CP:]
    Wo1, Wo2 = W_out[:D_CP], W_out[D_CP:]
    nh, nfin = HID * g, D_FIN * g
    k2 = nfin + 1 + 2 * D_CP * g
    mout = D_CP * g
    mtot = nh + mout
    ones_row = nfin
    cp0_base = nfin + 1
    cv_base = cp0_base + D_CP * g

    E = Wo1 @ W1p                      # (2, 12) cp0 contribution to pre
    r = b_rnn + b_out @ W1p            # (12,) ones-row weight (steady state)
    Wh = Wo2 @ W1p                     # (12, 12) h contribution to next pre

    w1 = np.zeros((nh, mtot), np.float32)
    w2 = np.zeros((k2, mtot), np.float32)
    w20 = np.zeros((k2, mtot), np.float32)
    for i in range(g):
        hsl = slice(HID * i, HID * (i + 1))
        osl = slice(nh + D_CP * i, nh + D_CP * (i + 1))
        w1[hsl, hsl] = Wh
        w1[hsl, osl] = Wo2
        fsl = slice(D_FIN * i, D_FIN * (i + 1))
        w2[fsl, hsl] = W1f
        w20[fsl, hsl] = W1f
        w2[ones_row, hsl] = r
        w20[ones_row, hsl] = b_rnn
        csl = slice(cp0_base + D_CP * i, cp0_base + D_CP * (i + 1))
        w2[csl, hsl] = E
        w20[csl, hsl] = W1p
        vsl = slice(cv_base + D_CP * i, cv_base + D_CP * (i + 1))
        w2[vsl, osl] = np.eye(D_CP, dtype=np.float32)
        w20[vsl, osl] = np.eye(D_CP, dtype=np.float32)
    return w1, w2, w20


def stage_inputs(cp0, fin, cvec, g=G, c=C, cols=COLS, t_steps=T):
    """Batch-major -> feature-major device layouts (b = ch*(g*cols)+gi*cols+j)."""
    bp = g * c * cols
    bc = cp0.shape[0]
    fin_p = np.zeros((bp, t_steps, D_FIN), np.float32)
    fin_p[:bc] = fin
    cp0_p = np.zeros((bp, D_CP), np.float32)
    cp0_p[:bc] = cp0
    cv_p = np.zeros((bp, D_CP), np.float32)
    cv_p[:bc] = cvec
    fin_d = np.ascontiguousarray(
        fin_p.reshape(c, g, cols, t_steps, D_FIN).transpose(3, 0, 1, 4, 2)
    ).reshape(t_steps, c, D_FIN * g, cols)
    # static fc rows: ones | cp0 | cvec  (k2 - nfin = 1 + 4g rows)
    xc_d = np.ones((c, 1 + 2 * D_CP * g, cols), np.float32)
    xc_d[:, 1 : 1 + D_CP * g, :] = (
        cp0_p.reshape(c, g, cols, D_CP).transpose(0, 1, 3, 2).reshape(c, D_CP * g, cols)
    )
    xc_d[:, 1 + D_CP * g :, :] = (
        cv_p.reshape(c, g, cols, D_CP).transpose(0, 1, 3, 2).reshape(c, D_CP * g, cols)
    )
    xc_d = np.tile(xc_d, (1, 1, 8))
    return fin_d, xc_d


def unstage_output(out_d, bc, g=G, c=C, cols=COLS, t_steps=T):
    bp = g * c * cols
    o = out_d.reshape(t_steps, c, g, D_CP, cols).transpose(1, 2, 4, 0, 3)
    o = np.ascontiguousarray(o).reshape(bp, t_steps, D_CP)[:bc]
    return np.arctanh(np.clip(o, -0.999999, 0.999999))


def kernel(control_point_input, finger_input, W_rnn, U_rnn, b_rnn, W_out, b_out):
    global LAST_RESULTS
    cp = np.asarray(control_point_input, np.float32)
    fin = np.asarray(finger_input, np.float32)
    W_rnn = np.asarray(W_rnn, np.float32)
    b_rnn = np.asarray(b_rnn, np.float32)
    W_out = np.asarray(W_out, np.float32)
    b_out = np.asarray(b_out, np.float32)

    cp0 = cp[:, 0, :]
    cvec = cp0 @ W_out[:D_CP] + b_out
    w1, w2, w20 = build_packed_weights(W_rnn, W_out, b_rnn, b_out)
    w1, w2, w20 = (x.astype(MM_NP) for x in (w1, w2, w20))

    nc = build_program()
    in_maps = []
    for m in range(NCORES):
        sl = slice(m * BC, (m + 1) * BC)
        fin_d, xc_d = stage_inputs(cp0[sl], fin[sl], cvec[sl])
        in_maps.append(
            {"fin": fin_d.astype(MM_NP, copy=False),
             "xc": xc_d.astype(MM_NP, copy=False),
             "w1": w1, "w2": w2, "w20": w20}
        )

    trace = bool(os.environ.get("DTB_TRACE"))
    res = run_bass_kernel_spmd(
        nc, in_maps, core_ids=list(range(NCORES)), trace=trace
    )
    LAST_RESULTS = res

    outs = [
        unstage_output(np.asarray(res.results[m]["out"], np.float32), BC)
        for m in range(NCORES)
    ]
    return np.concatenate(outs, axis=0)
